# revision 26
# baseline (speedup 1.0000x reference)
"""Trainium2 Bass kernel for nn_DeepCrossNetworkModel_Controller_hard.

Model: per-field embedding gather -> BatchNorm1d(F) (eval) -> controller
linear + softmax over fields -> top-k mask (renormalized) -> CrossNetwork(6)
+ MLP(2496->1024->512, BN+ReLU) -> concat -> linear -> sigmoid.

Strategy (data-parallel over 8 NeuronCores, 2048 rows each):
 - BN folded into the embedding table on host; table stored as bf16 256B
   rows with parity-dependent zero padding (even fields [0|row], odd
   fields [row|0]) so a contiguous 128-wide window at slot offset +64
   holds [even_feats | odd_feats] for one row chunk — one bf16 PE
   transpose yields a full feature-major k-tile.
 - top-k of softmax + renormalize == softmax restricted to top-k logits, so
   only the top-16 logits are ever exponentiated (max8 + match_replace x2).
 - CrossNetwork collapses algebraically: x_l = x0 * alpha_l + beta_l with
   beta_l a host constant vector; on device only U = x0 @ [cross_w; lin_w_a]
   (7 columns) plus a scalar recursion per row is needed.
 - x0 is written as scaled fp8e4 (scale folded into the mask-expand matrix);
   MLP0/MLP1 run as fp8 DoubleRow matmuls (2 k-tiles per instruction, 2x),
   with dequant scales folded into the ReLU activations. U also reads the
   fp8 x0. Controller, mask, h1 and the final dot stay bf16.
"""

import sys

if "/opt/trn_rl_repo" not in sys.path:
    sys.path.insert(0, "/opt/trn_rl_repo")

import ml_dtypes
import numpy as np

import concourse.bass as bass
import concourse.bacc as bacc
import concourse.mybir as mybir
import concourse.tile as tile
from concourse.bass_utils import run_bass_kernel_spmd
from concourse.masks import make_identity

# Problem constants (hardcoded per spec).
B, F, E, L = 16384, 39, 64, 6
VOCAB = 10000
D = F * E  # 2496
H0, H1 = 1024, 512
EPS = 1e-5
NCORES = 8
BPC = B // NCORES      # 2048 rows per core
BLK = 512              # batch block
NBLK = BPC // BLK      # 4
NCHUNK = BLK // 128    # 4 chunks of 128 rows per block
KT = 20                # feature k-tiles of 128 (D padded 2496 -> 2560)
KTP = KT // 2          # DoubleRow k-tile pairs for MLP0
NIDX = 2 * BLK         # indices per gather (field pair x 512 rows)
IDXW = NIDX // 16      # idx free width per gather (64)
M0 = H0 // 128         # 8
M1 = H1 // 128         # 4
KT1 = H0 // 128        # 8
KTP1 = KT1 // 2        # DoubleRow pairs for MLP1
NTAB = F * VOCAB + 1   # pair-table rows
PAIRS = NBLK // 2      # 2-block gather pairs
NIDX2 = 4 * BLK        # indices per paired gather (2 fields x 1024 rows)
IDXW2 = NIDX2 // 16    # 128

dt = mybir.dt
AF = mybir.ActivationFunctionType
OP = mybir.AluOpType
PM = mybir.MatmulPerfMode
bf16 = ml_dtypes.bfloat16
f8e4 = ml_dtypes.float8_e4m3  # TRN float8e4: max finite 240

_CACHE = {}


def _auto_scale(amax, target=120.0):
    return float(2.0 ** np.floor(np.log2(target / float(amax))))


def _dma_gather_128b(eng, out_ap, in_ap, idxs_ap, num_idxs, num_idxs_reg,
                     elem_size, elem_step, single_packet, queue_num):
    """dma_gather emitting elem_size*dtsize==128B fetches on a 256B stride.

    Replicates BassGpSimd.dma_gather's non-transpose emission without the
    256B elem_size assert (which the wrapper marks a transpose restriction);
    the descriptor stride stays a 256B multiple as hardware requires.
    """
    assert idxs_ap.dtype == dt.int16
    assert in_ap.ap[-1][1] == out_ap.ap[-1][1] == elem_size
    assert out_ap.ap[0][1] * out_ap.ap[1][1] == num_idxs
    assert in_ap.ap[0][0] == elem_step
    stride_bytes = elem_step * dt.size(in_ap.dtype)
    stride_256 = stride_bytes // 256
    assert stride_256 * 256 == stride_bytes and 0 < stride_256 < 256
    _in_ap = eng.lower_ap_dma(in_ap, for_custom_bir_dma=True)
    inst = eng.add_instruction(
        mybir.InstDMAGatherAnt(
            name=eng.bass.get_next_instruction_name(),
            ins=[
                *_in_ap,
                eng.lower_ap(idxs_ap),
                eng.lower_val_access(eng.to_reg(num_idxs_reg)),
            ],
            outs=[eng.lower_ap(out_ap)],
            transpose=False,
            num_idxs=num_idxs,
            elem_size=elem_size,
            stride_bytes_256=stride_256,
            gen_mode=0,
            single_packet=single_packet,
            queue_num=queue_num,
            sbuf_tokens_per_rank=0,
            sbuf_free_dim_per_rank=0,
            sbuf_free_dim_pad_per_rank=0,
            sbuf_byte_offset=0,
        )
    )
    return inst


def _build(k, v_consts, c0, scale0, scale1, udq, raw128, gp=False, passes=1,
           ablate=None, nq=4, sp=True, inter=True, unroll=False, gbufs=None,
           evac="dve"):
    """Build the per-core SPMD bass module."""
    nc = bacc.Bacc("TRN2", target_bir_lowering=False, debug=False,
                   num_devices=NCORES, num_swdge_queues=nq)

    if gp:
        assert raw128
        idxs_d = nc.declare_dram_parameter("idxs", [PAIRS, 128, KT * IDXW2], dt.int16, isOutput=False)
    else:
        idxs_d = nc.declare_dram_parameter("idxs", [NBLK, 128, KT * IDXW], dt.int16, isOutput=False)
    tab_d = nc.declare_dram_parameter("tab", [NTAB, 2 * E], dt.bfloat16, isOutput=False)
    wc_d = nc.declare_dram_parameter("wc", [128, KT * F], dt.bfloat16, isOutput=False)
    w0_d = nc.declare_dram_parameter("w0", [128, KT * M0 * 128], dt.float8e4, isOutput=False)
    w1_d = nc.declare_dram_parameter("w1", [128, KT1 * M1 * 128], dt.float8e4, isOutput=False)
    wu_d = nc.declare_dram_parameter("wu", [128, KT * 7], dt.float8e4, isOutput=False)
    s_d = nc.declare_dram_parameter("s", [F, KT * 128], dt.bfloat16, isOutput=False)
    lw2_d = nc.declare_dram_parameter("lw2", [128, M1], dt.bfloat16, isOutput=False)
    b0_d = nc.declare_dram_parameter("b0", [128, M0], dt.float32, isOutput=False)
    b1_d = nc.declare_dram_parameter("b1", [128, M1], dt.float32, isOutput=False)
    out_d = nc.declare_dram_parameter("out", [BPC], dt.float32, isOutput=True)

    rounds = -(-k // 8)  # ceil(k/8) match_replace rounds
    al_tiles = {}

    with tile.TileContext(nc) as tc:
        with (
            tc.tile_pool(name="const", bufs=1) as cpool,
            tc.tile_pool(name="big", bufs=1) as bigp,
            tc.tile_pool(name="fm2", bufs=2) as fm2p,
            tc.tile_pool(name="gat", bufs=gbufs or (34 if gp else 22)) as gatp,
            tc.tile_pool(name="idx", bufs=2) as idxp,
            tc.tile_pool(name="scr", bufs=8) as scr,
            tc.tile_pool(name="pst", bufs=3, space="PSUM") as pst,
            tc.tile_pool(name="psb", bufs=3, space="PSUM") as psb,
            tc.tile_pool(name="pss", bufs=2, space="PSUM") as pss,
        ):
            # ---- identities + small constants first (cheap, unblock PE) ----
            idf = cpool.tile([128, 128], dt.float32)
            make_identity(nc, idf[:])
            idb = cpool.tile([128, 128], dt.bfloat16)
            make_identity(nc, idb[:])

            wc_sb = cpool.tile([128, KT * F], dt.bfloat16)
            nc.sync.dma_start(wc_sb[:], wc_d[:])
            s_sb = cpool.tile([F, KT * 128], dt.bfloat16)
            nc.sync.dma_start(s_sb[:], s_d[:])
            wu_sb = cpool.tile([128, KT * 7], dt.float8e4)
            nc.sync.dma_start(wu_sb[:], wu_d[:])
            lw2_sb = cpool.tile([128, M1], dt.bfloat16)
            nc.sync.dma_start(lw2_sb[:], lw2_d[:])
            b0_sb = cpool.tile([128, M0], dt.float32)
            nc.sync.dma_start(b0_sb[:], b0_d[:])
            b1_sb = cpool.tile([128, M1], dt.float32)
            nc.sync.dma_start(b1_sb[:], b1_d[:])
            w0_sb = cpool.tile([128, KT * M0 * 128], dt.float8e4)
            for q in range(2):
                qs = KT * M0 * 128 // 2
                nc.sync.dma_start(w0_sb[:, q * qs : (q + 1) * qs],
                                  w0_d[:, q * qs : (q + 1) * qs])
            w1_sb = cpool.tile([128, KT1 * M1 * 128], dt.float8e4)
            nc.sync.dma_start(w1_sb[:], w1_d[:])

            # ---- persistent activations (one block in flight) ----
            flat_fm = fm2p.tile([128, KT * BLK], dt.bfloat16)
            x0_fm = bigp.tile([128, KT * BLK], dt.float8e4)
            h0_fm = bigp.tile([128, M0 * BLK], dt.float8e4)
            h1_fm = bigp.tile([128, M1 * BLK], dt.bfloat16)
            mask_fm = bigp.tile([F, BLK], dt.bfloat16)
            p_sb = bigp.tile([128, BPC // 128], dt.float32)

            nreg = nc.gpsimd.to_reg(NIDX2 if gp else NIDX)
            if ablate in ("gather", "null", "ctl", "nomlp"):
                nc.vector.memset(p_sb[:], 0.5)
            if ablate == "compute":
                nc.vector.memset(flat_fm[:], 0.001)
            do_xu = ablate not in ("ctl",)          # expand/x0/U
            do_mlp = ablate not in ("ctl", "nomlp")  # MLP0/MLP1/r/p

            def one_pass():
                gt = {}

                def emit_gathers(blk):
                    if gp:
                        idx_sb = idxp.tile([128, KT * IDXW2], dt.int16)
                    else:
                        idx_sb = idxp.tile([128, KT * IDXW], dt.int16)
                    nc.sync.dma_start(idx_sb[:], idxs_d[blk, :, :])
                    niw = IDXW2 if gp else IDXW
                    nidx = NIDX2 if gp else NIDX
                    for g in range(KT):
                        lo = 2 * g * VOCAB
                        hi = min(lo + 2 * VOCAB + 1, NTAB)
                        if raw128:
                            gtile = gatp.tile([128, nidx // 128 * E],
                                              dt.bfloat16, tag="g")
                            _dma_gather_128b(
                                nc.gpsimd,
                                out_ap=gtile[:].rearrange("p (a e) -> p a e",
                                                          e=E),
                                in_ap=tab_d[lo:hi, 0:E],
                                idxs_ap=idx_sb[:, g * niw : (g + 1) * niw],
                                num_idxs=nidx,
                                num_idxs_reg=nreg,
                                elem_size=E,
                                elem_step=2 * E,
                                single_packet=sp,
                                queue_num=g % nq,
                            )
                        else:
                            gtile = gatp.tile([128, NIDX // 128 * 2 * E],
                                              dt.bfloat16, tag="g")
                            nc.gpsimd.dma_gather(
                                out_ap=gtile[:].rearrange("p (a e) -> p a e",
                                                          e=2 * E),
                                in_ap=tab_d[lo:hi, :],
                                idxs_ap=idx_sb[:, g * IDXW : (g + 1) * IDXW],
                                num_idxs=NIDX,
                                num_idxs_reg=nreg,
                                elem_size=2 * E,
                                single_packet=sp,
                                queue_num=g % nq,
                            )
                        gt[(blk, g)] = gtile

                def emit_transposes(blk, gs, ge):
                    # bf16 PE transpose; window spans
                    # [even-field feats | odd-field feats] of one chunk
                    pr, beta = divmod(blk, 2)
                    for g in range(gs, ge):
                        if gp:
                            gtile = gt.pop((pr, g)) if beta else gt[(pr, g)]
                        else:
                            gtile = gt.pop((blk, g))
                        tp = pst.tile([128, BLK], dt.bfloat16, space="PSUM",
                                      tag="t")
                        for c in range(NCHUNK):
                            if gp:
                                win = (beta * 4 + c) * 128
                            else:
                                win = (c * 128 if raw128
                                       else 256 * c + 64)
                            nc.tensor.transpose(
                                out=tp[:, c * 128 : (c + 1) * 128],
                                in_=gtile[:, win : win + 128],
                                identity=idb[:],
                            )
                        if evac == "act":
                            nc.scalar.activation(
                                flat_fm[:, g * BLK : (g + 1) * BLK], tp[:],
                                AF.Copy)
                        else:
                            nc.vector.tensor_copy(
                                flat_fm[:, g * BLK : (g + 1) * BLK], tp[:])

                do_g = ablate not in ("compute", "null")
                do_c = ablate not in ("gather", "null")
                if do_g:
                    emit_gathers(0)
                    if do_c:
                        emit_transposes(0, 0, KT)

                for blk in range(NBLK):
                    if gp:
                        if do_g and blk % 2 == 0 and blk // 2 + 1 < PAIRS:
                            emit_gathers(blk // 2 + 1)
                    elif do_g and blk + 1 < NBLK:
                        emit_gathers(blk + 1)
                    if not do_c:
                        continue
                    nxt = do_g and blk + 1 < NBLK and inter
                    if do_g and blk + 1 < NBLK and not inter:
                        emit_transposes(blk + 1, 0, KT)

                    # ---- controller + top-k mask (per 128-row chunk) ----
                    for c in range(NCHUNK):
                        z = pss.tile([128, 64], dt.float32, space="PSUM", tag="s")
                        for kt in range(KT):
                            nc.tensor.matmul(
                                z[:, :F],
                                lhsT=flat_fm[:, kt * BLK + c * 128 : kt * BLK + (c + 1) * 128],
                                rhs=wc_sb[:, kt * F : (kt + 1) * F],
                                start=(kt == 0), stop=(kt == KT - 1),
                            )
                        mx = scr.tile([128, 8], dt.float32, tag="mx")
                        nm = scr.tile([128, 1], dt.float32, tag="nm")
                        zap = scr.tile([128, F], dt.float32, tag="zap")
                        zap2 = scr.tile([128, F], dt.float32, tag="zap2")
                        esb = scr.tile([128, F], dt.float32, tag="esb")
                        ssum = scr.tile([128, 1], dt.float32, tag="ssum")
                        rcp = scr.tile([128, 1], dt.float32, tag="rcp")
                        mbm = scr.tile([128, F], dt.bfloat16, tag="mbm")
                        src = z[:, :F]
                        outs = [zap[:], zap2[:]]
                        for r in range(rounds):
                            nc.vector.max(out=mx[:], in_=src)
                            if r == 0:
                                nc.vector.tensor_scalar(
                                    nm[:], mx[:, 0:1], -1.0, None, op0=OP.mult)
                            if r == rounds - 1 and k - 8 * r < 8:
                                nc.vector.memset(mx[:, k - 8 * r :], -1e30)
                            nc.vector.match_replace(
                                out=outs[r % 2], in_to_replace=mx[:],
                                in_values=src, imm_value=-1e30)
                            src = outs[r % 2]
                        zfin = outs[(rounds - 1) % 2]
                        nc.scalar.activation(esb[:], z[:, :F], AF.Exp,
                                             bias=nm[:, 0:1], scale=1.0)
                        nc.vector.tensor_scalar(zfin, zfin, -1e30, None,
                                                op0=OP.is_equal)
                        nc.vector.tensor_tensor(esb[:], esb[:], zfin, op=OP.mult)
                        nc.vector.reduce_sum(ssum[:], esb[:],
                                             axis=mybir.AxisListType.X)
                        nc.vector.reciprocal(rcp[:], ssum[:])
                        nc.vector.tensor_scalar(mbm[:], esb[:], rcp[:, 0:1],
                                                None, op0=OP.mult)
                        mt = pst.tile([128, BLK], dt.bfloat16, space="PSUM",
                                      tag="t")
                        nc.tensor.transpose(out=mt[:F, :128], in_=mbm[:],
                                            identity=idb[:])
                        nc.vector.tensor_copy(
                            mask_fm[:, c * 128 : (c + 1) * 128], mt[:F, :128])

                    if not do_xu:
                        if nxt:
                            emit_transposes(blk + 1, 0, KT)
                        continue
                    # ---- expand mask*SX to features, apply -> x0 (fp8) ----
                    for kt in range(KT):
                        ex = psb.tile([128, BLK], dt.float32, space="PSUM",
                                      tag="b")
                        nc.tensor.matmul(
                            ex[:], lhsT=s_sb[:, kt * 128 : (kt + 1) * 128],
                            rhs=mask_fm[:], start=True, stop=True)
                        nc.vector.tensor_tensor(
                            x0_fm[:, kt * BLK : (kt + 1) * BLK],
                            flat_fm[:, kt * BLK : (kt + 1) * BLK], ex[:],
                            op=OP.mult)

                    # ---- U = x0 @ [cross_w; lin_w_a]  (batch-major out) ----
                    for c in range(NCHUNK):
                        u = pss.tile([128, 64], dt.float32, space="PSUM", tag="s")
                        for kt in range(KT):
                            nc.tensor.matmul(
                                u[:, :7],
                                lhsT=x0_fm[:, kt * BLK + c * 128 : kt * BLK + (c + 1) * 128],
                                rhs=wu_sb[:, kt * 7 : kt * 7 + 7],
                                start=(kt == 0), stop=(kt == KT - 1),
                            )
                        usb = scr.tile([128, 8], dt.float32, tag="usb")
                        nc.vector.tensor_scalar(usb[:, :7], u[:, :7], udq,
                                                None, op0=OP.mult)
                        al = scr.tile([128, 1], dt.float32, tag="al")
                        t1 = scr.tile([128, 1], dt.float32, tag="t1")
                        nc.vector.tensor_scalar(al[:], usb[:, 0:1],
                                                1.0 + v_consts[0], None, op0=OP.add)
                        for l in range(1, L):
                            nc.vector.tensor_scalar(t1[:], usb[:, l : l + 1],
                                                    1.0, None, op0=OP.add)
                            nc.vector.tensor_tensor(al[:], al[:], t1[:],
                                                    op=OP.mult)
                            if v_consts[l] != 0.0:
                                nc.vector.tensor_scalar(al[:], al[:],
                                                        v_consts[l], None,
                                                        op0=OP.add)
                        nc.vector.tensor_tensor(al[:], al[:], usb[:, 6:7],
                                                op=OP.mult)
                        al_tiles[(blk, c)] = al

                    if not do_mlp:
                        for c in range(NCHUNK):
                            al_tiles.pop((blk, c))
                        if nxt:
                            emit_transposes(blk + 1, 0, KT)
                        continue
                    # ---- MLP layer 0 (fp8 DoubleRow), next block's
                    # ---- transposes interleaved ----
                    for m in range(M0):
                        hp = psb.tile([128, BLK], dt.float32, space="PSUM",
                                      tag="b")
                        for p in range(KTP):
                            off = ((p * M0 + m) * 2) * 128
                            nc.tensor.matmul(
                                hp[:],
                                lhsT=w0_sb[:, off : off + 256].rearrange(
                                    "k (two m) -> k two m", two=2),
                                rhs=x0_fm[:, 2 * p * BLK : (2 * p + 2) * BLK].rearrange(
                                    "k (two n) -> k two n", two=2),
                                start=(p == 0), stop=(p == KTP - 1),
                                perf_mode=PM.DoubleRow,
                            )
                        nc.scalar.activation(h0_fm[:, m * BLK : (m + 1) * BLK],
                                             hp[:], AF.Relu,
                                             bias=b0_sb[:, m : m + 1],
                                             scale=scale0)
                        if nxt:
                            # ~3 field-pair transposes between m-tiles
                            gs = m * 3
                            emit_transposes(blk + 1, gs, min(gs + 3, KT))

                    # ---- MLP layer 1 (fp8 DoubleRow) ----
                    for m in range(M1):
                        hp = psb.tile([128, BLK], dt.float32, space="PSUM",
                                      tag="b")
                        for p in range(KTP1):
                            off = ((p * M1 + m) * 2) * 128
                            nc.tensor.matmul(
                                hp[:],
                                lhsT=w1_sb[:, off : off + 256].rearrange(
                                    "k (two m) -> k two m", two=2),
                                rhs=h0_fm[:, 2 * p * BLK : (2 * p + 2) * BLK].rearrange(
                                    "k (two n) -> k two n", two=2),
                                start=(p == 0), stop=(p == KTP1 - 1),
                                perf_mode=PM.DoubleRow,
                            )
                        nc.scalar.activation(h1_fm[:, m * BLK : (m + 1) * BLK],
                                             hp[:], AF.Relu,
                                             bias=b1_sb[:, m : m + 1],
                                             scale=scale1)
                    if nxt:
                        emit_transposes(blk + 1, M0 * 3, KT)

                    # ---- r = h1 . lin_w_b ; p = sigmoid(alpha*q + r + c0) ----
                    for c in range(NCHUNK):
                        rp = pss.tile([128, 64], dt.float32, space="PSUM", tag="s")
                        for kt in range(M1):
                            nc.tensor.matmul(
                                rp[:, :1],
                                lhsT=h1_fm[:, kt * BLK + c * 128 : kt * BLK + (c + 1) * 128],
                                rhs=lw2_sb[:, kt : kt + 1],
                                start=(kt == 0), stop=(kt == M1 - 1),
                            )
                        al = al_tiles.pop((blk, c))
                        t2 = scr.tile([128, 1], dt.float32, tag="t2")
                        nc.vector.tensor_tensor(t2[:], al[:], rp[:, 0:1],
                                                op=OP.add)
                        nc.scalar.activation(
                            p_sb[:, blk * NCHUNK + c : blk * NCHUNK + c + 1],
                            t2[:], AF.Sigmoid, bias=float(c0), scale=1.0)

                # ---- transpose p [128, 16] -> [16, 128] and store ----
                if ablate in ("gather", "null"):
                    nc.sync.dma_start(out_d[:].rearrange("(a b) -> a b", b=16),
                                      p_sb[:])
                    return
                ptp = pss.tile([128, 128], dt.float32, space="PSUM", tag="s")
                nc.tensor.transpose(out=ptp[: BPC // 128, :], in_=p_sb[:],
                                    identity=idf[:])
                pout = cpool.tile([BPC // 128, 128], dt.float32)
                nc.vector.tensor_copy(pout[:], ptp[: BPC // 128, :])
                nc.sync.dma_start(out_d[:].rearrange("(a b) -> a b", b=128),
                                  pout[:])

            if passes == 1:
                one_pass()
            elif unroll:
                for _ in range(passes):
                    one_pass()
            else:
                with tc.For_i(0, passes, 1):
                    one_pass()

    nc.compile()
    return nc


RAW128 = True
# GP=True (2048-idx paired gathers) passes walrus+CoreSim but fails on real
# hardware — the gather ucode appears to cap num_idxs at 1024. Keep off.
GP = False


def _prep_host(inputs, raw128=None, gp=None):
    """Host-side preprocessing -> per-core input maps."""
    if raw128 is None:
        raw128 = RAW128
    if gp is None:
        gp = GP and raw128
    x = np.asarray(inputs["x"]).astype(np.int64)
    tab = np.asarray(inputs["emb_table"], dtype=np.float32)
    k = int(np.asarray(inputs["k"]))

    s_f = (np.asarray(inputs["bn_gamma"], np.float64)
           / np.sqrt(np.asarray(inputs["bn_var"], np.float64) + EPS))
    t_f = np.asarray(inputs["bn_beta"], np.float64) - np.asarray(
        inputs["bn_mean"], np.float64) * s_f
    tab_bn = (tab.astype(np.float64) * np.repeat(s_f, VOCAB)[:, None]
              + np.repeat(t_f, VOCAB)[:, None]).astype(np.float32)
    const_row = np.zeros((1, E), np.float32)
    const_row[0, 0] = 1.0
    # row F*VOCAB = bias feature (odd-parity pseudo-field 39)
    tab_ext = np.concatenate([tab_bn, const_row], 0).astype(bf16)
    tab_h = np.zeros((NTAB, 2 * E), bf16)
    if raw128:
        # 256B-strided rows, data in the first 128B (fetched half)
        tab_h[:, :E] = tab_ext
    else:
        # 256B rows with parity-dependent halves: even fields [0|row],
        # odd fields [row|0]
        parity = (np.arange(NTAB) // VOCAB) % 2
        tab_h[parity == 0, E:] = tab_ext[parity == 0]
        tab_h[parity == 1, :E] = tab_ext[parity == 1]

    amax_tab = float(np.abs(tab_ext.astype(np.float32)).max())
    sx = _auto_scale(amax_tab)          # x0 fp8 scale (mask <= 1)
    sh0 = 8.0                           # h0 fp8 scale (h0 ~< 8; 240/8 = 30)

    # controller weights, padded D 2496 -> 2560 with bias as ones-feature row
    wc = np.zeros((KT * 128, F), np.float32)
    wc[:D] = np.asarray(inputs["ctrl_w"], np.float32)
    wc[D] = np.asarray(inputs["ctrl_b"], np.float32)
    wc_h = np.ascontiguousarray(
        wc.reshape(KT, 128, F).transpose(1, 0, 2).reshape(128, KT * F)).astype(bf16)

    # MLP0 with BN scale folded into columns; fp8 DoubleRow layout
    g0 = (np.asarray(inputs["mlp_g0"], np.float64)
          / np.sqrt(np.asarray(inputs["mlp_v0"], np.float64) + EPS))
    w0 = np.zeros((KT * 128, H0), np.float32)
    w0[:D] = np.asarray(inputs["mlp_w0"], np.float32) * g0[None, :].astype(np.float32)
    sw0 = _auto_scale(np.abs(w0).max())
    b0 = ((np.asarray(inputs["mlp_b0"], np.float64)
           - np.asarray(inputs["mlp_m0"], np.float64)) * g0
          + np.asarray(inputs["mlp_be0"], np.float64)).astype(np.float32)
    w0_h = np.ascontiguousarray(
        (w0 * sw0).reshape(KTP, 2, 128, M0, 128).transpose(2, 0, 3, 1, 4)
        .reshape(128, KT * M0 * 128)).astype(f8e4)
    b0_h = np.ascontiguousarray((b0 * sh0).reshape(M0, 128).T)
    scale0 = float(sh0 / (sx * sw0))

    g1 = (np.asarray(inputs["mlp_g1"], np.float64)
          / np.sqrt(np.asarray(inputs["mlp_v1"], np.float64) + EPS))
    w1 = np.asarray(inputs["mlp_w1"], np.float32) * g1[None, :].astype(np.float32)
    sw1 = _auto_scale(np.abs(w1).max())
    b1 = ((np.asarray(inputs["mlp_b1"], np.float64)
           - np.asarray(inputs["mlp_m1"], np.float64)) * g1
          + np.asarray(inputs["mlp_be1"], np.float64)).astype(np.float32)
    w1_h = np.ascontiguousarray(
        (w1 * sw1).reshape(KTP1, 2, 128, M1, 128).transpose(2, 0, 3, 1, 4)
        .reshape(128, KT1 * M1 * 128)).astype(f8e4)
    b1_h = np.ascontiguousarray(b1.reshape(M1, 128).T)
    scale1 = float(1.0 / (sh0 * sw1))

    # U weights: 6 cross rows + lin_w[:D], padded; fp8
    cross_w = np.asarray(inputs["cross_w"], np.float32)
    cross_b = np.asarray(inputs["cross_b"], np.float64)
    lin_w = np.asarray(inputs["lin_w"], np.float32)
    wu = np.zeros((KT * 128, 7), np.float32)
    wu[:D, :L] = cross_w.T
    wu[:D, 6] = lin_w[:D]
    swu = _auto_scale(np.abs(wu).max())
    wu_h = np.ascontiguousarray(
        (wu * swu).reshape(KT, 128, 7).transpose(1, 0, 2)
        .reshape(128, KT * 7)).astype(f8e4)
    udq = float(1.0 / (sx * swu))

    # expand matrix S [F, KT*128] carrying the x0 fp8 scale
    s = np.zeros((F, KT * 128), np.float32)
    feat = np.arange(KT * 128)
    valid = feat < D
    s[feat[valid] // E, feat[valid]] = sx
    s_h = s.astype(bf16)

    lw2_h = np.ascontiguousarray(lin_w[D:].reshape(M1, 128).T).astype(bf16)

    # cross-collapse constants: v_l = beta_l . w_l ; c0 = beta_6 . lin_w_a + b
    beta = np.zeros(D, np.float64)
    v = np.zeros(L, np.float64)
    for l in range(L):
        v[l] = beta @ cross_w[l].astype(np.float64)
        beta = beta + cross_b[l]
    c0 = float(beta @ lin_w[:D].astype(np.float64)
               + float(np.asarray(inputs["lin_b"]).ravel()[0]))
    v_consts = tuple(float(t) for t in v)

    in_maps = []
    for ci in range(NCORES):
        xs = x[ci * BPC : (ci + 1) * BPC]  # [2048, 39]
        if gp:
            # paired gathers: 2 blocks of rows per gather instruction
            idxs = np.zeros((PAIRS, 128, KT * IDXW2), np.int16)
            for pr in range(PAIRS):
                for g in range(KT):
                    jj = np.zeros(NIDX2, np.int64)
                    for beta in range(2):
                        blk = pr * 2 + beta
                        for c in range(NCHUNK):
                            for fr in range(2):
                                a = (beta * NCHUNK + c) * 2 + fr
                                f = 2 * g + fr
                                rows = xs[blk * BLK + c * 128 : blk * BLK + (c + 1) * 128, f] \
                                    if f < F else np.zeros(128, np.int64)
                                jj[a * 128 : (a + 1) * 128] = rows + fr * VOCAB
                    assert 0 <= jj.min() and jj.max() <= 2 * VOCAB
                    wrapped = jj.reshape(IDXW2, 16).T.astype(np.int16)
                    idxs[pr, :, g * IDXW2 : (g + 1) * IDXW2] = np.tile(wrapped, (8, 1))
        else:
            idxs = np.zeros((NBLK, 128, KT * IDXW), np.int16)
            for blk in range(NBLK):
                for g in range(KT):
                    # J[a*128 + p], a = c*2 + f_rel (chunk-major)
                    jj = np.zeros(NIDX, np.int64)
                    for c in range(NCHUNK):
                        for fr in range(2):
                            a = c * 2 + fr
                            f = 2 * g + fr
                            rows = xs[blk * BLK + c * 128 : blk * BLK + (c + 1) * 128, f] \
                                if f < F else np.zeros(128, np.int64)
                            jj[a * 128 : (a + 1) * 128] = rows + fr * VOCAB
                    assert 0 <= jj.min() and jj.max() <= 2 * VOCAB
                    # wrap: index j at [j % 16, j // 16], replicated x8
                    wrapped = jj.reshape(IDXW, 16).T.astype(np.int16)  # [16, IDXW]
                    idxs[blk, :, g * IDXW : (g + 1) * IDXW] = np.tile(wrapped, (8, 1))
        in_maps.append({
            "idxs": idxs,
            "tab": tab_h,
            "wc": wc_h,
            "w0": w0_h,
            "w1": w1_h,
            "wu": wu_h,
            "s": s_h,
            "lw2": lw2_h,
            "b0": b0_h,
            "b1": b1_h,
        })
    return in_maps, (k, v_consts, c0, scale0, scale1, udq, raw128, gp)


def _get_nc(key, **kw):
    ck = (key, tuple(sorted(kw.items())))
    if ck not in _CACHE:
        _CACHE[ck] = _build(*key, **kw)
    return _CACHE[ck]


def kernel(**inputs) -> np.ndarray:
    in_maps, key = _prep_host(inputs)
    nc = _get_nc(key)
    res = run_bass_kernel_spmd(nc, in_maps, core_ids=list(range(NCORES)))
    out = np.concatenate([res.results[i]["out"] for i in range(NCORES)])
    return out.astype(np.float32)


def run_traced(**inputs):
    """Like kernel() but with tracing enabled; returns (out, results)."""
    in_maps, key = _prep_host(inputs)
    nc = _get_nc(key)
    res = run_bass_kernel_spmd(nc, in_maps, core_ids=list(range(NCORES)),
                               trace=True)
    out = np.concatenate([res.results[i]["out"] for i in range(NCORES)])
    return out.astype(np.float32), res
